# revision 3
# baseline (speedup 1.0000x reference)
"""Trainium2 Bass kernel for nn_DABConv (deformable attention-ish conv).

Data-parallel over batch: 8 samples -> 8 NeuronCores, one sample per core.

v2: the kernel is gather-stream-bound (288 indirect DMAs x ~1.1us serialized
on GpSimd SWDGE desc-gen).  Everything else is arranged to hide under that
stream:
  - x loaded+padded via ONE gpsimd cast-DMA (f32->bf16) into xp.
  - channel-last image xcl built with ONE DMA-transpose (XBAR), not PE.
  - offset/modulator conv output transposed to position-major with ONE
    DMA-transpose (omb padded to 128 partitions).
  - per (wave,tap): 16x 1KB-patch indirect gathers -> 3 DVE combine ops
    -> ONE DMA-transpose (samp -> rhsT, replaces 16 PE transposes + 16
    PSUM->SBUF copies) -> 16 accumulating def-conv matmuls.
  - fused 1x1 conv + output DMA emitted per wave so the tail is short.

Patch gather trick (from v1): x2cl[(r,c)] = [x_cl[r,c] | x_cl[r+1,c]] bf16 in
DRAM, so one 1KB gathered element at row iy*68+ix = the full 2x2 bilinear
patch [v00|v10|v01|v11] x 128ch.
"""

import numpy as np
import ml_dtypes
from contextlib import ExitStack

import concourse.bass as bass
import concourse.bacc as bacc
import concourse.mybir as mybir
from concourse.tile import TileContext
from concourse.bass_utils import run_bass_kernel_spmd

AF = mybir.ActivationFunctionType
OP = mybir.AluOpType
F32 = mybir.dt.float32
BF16 = mybir.dt.bfloat16
NPBF = ml_dtypes.bfloat16

P = 128
H = 64
HP = 68          # padded image side (pad=2 each side)
NP = H * H       # 4096 output positions
NPAD = HP * HP   # 4624 padded positions
NBLK = 37        # ceil(4624/128)
XPW = NBLK * P   # 4736 (xp free width incl. zero tail for the transpose)
K = 9
MAGIC = 12582912.0  # 2**23 + 2**22: float32 round-to-int trick
NW = 2           # waves over position blocks (PSUM capacity)
WBLK = 16        # 128-position blocks per wave
N_CORES = 8


def _r3(ap, inner):
    return ap.rearrange("p (a b) -> p a b", b=inner)


def build_nc():
    nc = bacc.Bacc("TRN2", target_bir_lowering=False, debug=False)

    x_d = nc.dram_tensor("x", [P, NP], F32, kind="ExternalInput")
    wstd_d = nc.dram_tensor("wstd", [K, P, P], BF16, kind="ExternalInput")
    wom_d = nc.dram_tensor("wom", [K, P, 32], BF16, kind="ExternalInput")
    wdef_d = nc.dram_tensor("wdef", [K, P, P], BF16, kind="ExternalInput")
    wfus_d = nc.dram_tensor("wfus", [2, P, P], BF16, kind="ExternalInput")
    bfus_d = nc.dram_tensor("bfus", [P, 1], F32, kind="ExternalInput")
    bom_d = nc.dram_tensor("bom", [32, 1], F32, kind="ExternalInput")
    yb_d = nc.dram_tensor("yb", [P, K * 32], F32, kind="ExternalInput")
    xb_d = nc.dram_tensor("xb", [P, K * 32], F32, kind="ExternalInput")
    out_d = nc.dram_tensor("out", [P, NP], F32, kind="ExternalOutput")
    # internal scratch: channel-last row-pair image; row j = padded pos
    # (r,c) holds [x_cl[r,c] | x_cl[r+1,c]] x 128ch bf16; a 512-elem gather
    # at row j spans rows j, j+1 = the full 2x2 bilinear patch.
    x2_d = nc.dram_tensor("x2cl", [NBLK * 128, 256], BF16)

    with TileContext(nc) as tc, ExitStack() as top:
        const = top.enter_context(tc.tile_pool(name="const", bufs=1))
        main = top.enter_context(tc.tile_pool(name="main", bufs=1))

        # ---- const loads ----
        wstd = const.tile([P, K * P], BF16, tag="wstd", name="wstd")
        nc.sync.dma_start(_r3(wstd, P), wstd_d[:, :, :].transpose([1, 0, 2]))
        wom = const.tile([P, K * 32], BF16, tag="wom", name="wom")
        nc.sync.dma_start(_r3(wom, 32), wom_d[:, :, :].transpose([1, 0, 2]))
        wdef = const.tile([P, K * P], BF16, tag="wdef", name="wdef")
        nc.sync.dma_start(_r3(wdef, P), wdef_d[:, :, :].transpose([1, 0, 2]))
        wfus = const.tile([P, 2 * P], BF16, tag="wfus", name="wfus")
        nc.sync.dma_start(_r3(wfus, P), wfus_d[:, :, :].transpose([1, 0, 2]))
        bfus = const.tile([P, 1], F32, tag="bfus", name="bfus")
        nc.sync.dma_start(bfus[:, :], bfus_d[:, :])
        bom = const.tile([32, 1], F32, tag="bom", name="bom")
        nc.sync.dma_start(bom[:, :], bom_d[:, :])
        yb = const.tile([P, K * 32], F32, tag="yb", name="yb")
        nc.sync.dma_start(yb[:, :], yb_d[:, :])
        xb = const.tile([P, K * 32], F32, tag="xb", name="xb")
        nc.sync.dma_start(xb[:, :], xb_d[:, :])

        # ---- long-lived tiles ----
        xp = main.tile([P, XPW], BF16, tag="xp", name="xp")
        xstd = main.tile([P, NP], BF16, tag="xstd", name="xstd")
        pmT = main.tile([P, 32 * P], BF16, tag="pmT", name="pmT")  # [p, b, ch]
        idx32 = main.tile([P, K * 32], mybir.dt.int32, tag="idx32", name="idx32")
        # corner scales, corner-interleaved + pair-duplicated:
        # col = (k*32+b)*8 + corner*2 + {0,1}; corner order matches the
        # gathered patch layout [v00 | v10 | v01 | v11].
        sall = main.tile([P, K * 32 * 8], BF16, tag="sall", name="sall")
        xdef = main.tile([P, NP], BF16, tag="xdef", name="xdef")

        # ================= phase A: padded bf16 image (cast-DMA) ========
        xpv = _r3(xp[:, 0:NPAD], HP)  # [128, 68, 68]
        nc.vector.memset(xp[:, 0 : 2 * HP], 0.0)                   # top rows
        nc.vector.memset(xp[:, 66 * HP : NPAD], 0.0)               # bottom rows
        nc.vector.memset(xpv[:, 2:66, 0:2], 0.0)                   # left cols
        nc.vector.memset(xpv[:, 2:66, 66:68], 0.0)                 # right cols
        nc.vector.memset(xp[:, NPAD:XPW], 0.0)                     # tail
        nc.gpsimd.dma_start(xpv[:, 2:66, 2:66], _r3(x_d[:, :], H))

        # ============= phase B: channel-last image + row-pair DRAM ======
        with tc.tile_pool(name="ph_b", bufs=1) as pb:
            xcl = pb.tile([P, NBLK, P], BF16, tag="xcl", name="xcl")
            # xcl[j, b, i] = xp[i, b*128 + j]  (one XBAR transpose)
            nc.sync.dma_start(xcl[:, :, :], xp[:, :], transpose=True)
            x2r = x2_d[:, :].rearrange("(b p) c -> p b c", p=P)  # [128, 37, 256]
            xclr = xcl[:, :, :]
            # plane 0: entry (r,c) <- x_cl[r,c]
            nc.sync.dma_start(x2r[:, :, 0:P], xclr)
            # plane 1: entry (r,c) <- x_cl[r+1,c]  (source shifted by 68 pos)
            nc.sync.dma_start(x2r[0:60, :, P:256], xclr[68:128, :, :])
            nc.sync.dma_start(x2r[60:128, 0 : NBLK - 1, P:256], xclr[0:68, 1:NBLK, :])

        # ================= phase C: offset/modulator conv ===============
        def conv_rhs(n, ki, kj):
            base = (8 * n + ki + 1) * HP
            v = xp[:, base : base + 8 * HP]
            return _r3(v, HP)[:, :, kj + 1 : kj + 1 + H]

        ps_conv = top.enter_context(tc.tile_pool(name="ps_conv", bufs=2, space="PSUM"))
        with tc.tile_pool(name="ph_c", bufs=1) as pc:
            omb = pc.tile([P, NP], BF16, tag="omb", name="omb")  # rows 0..31 valid
            for n in range(8):
                ps = ps_conv.tile([P, 512], F32, tag="ps_c", name="ps_c")
                for k in range(K):
                    nc.tensor.matmul(
                        ps[:32, :], wom[:, k * 32 : (k + 1) * 32],
                        conv_rhs(n, k // 3, k % 3),
                        start=(k == 0), stop=(k == K - 1),
                    )
                nc.scalar.activation(
                    omb[:32, n * 512 : (n + 1) * 512], ps[:32, :], AF.Identity,
                    bias=bom[:, :],
                )
            # ===== phase D: ONE XBAR transpose to position-major ========
            # pmT[j, b, ch] = omb[ch, b*128 + j]; ch>=32 is garbage, never read
            nc.scalar.dma_start(pmT[:, :].rearrange("p (b c) -> p b c", c=P),
                                omb[:, :], transpose=True)

        # ================= phase E: index & weight math =================
        pm3 = pmT.rearrange("p (b c) -> p b c", c=P)  # [128, 32, 128]
        dyv = pm3[:, :, 0:18:2].transpose([0, 2, 1])   # [p, 9, 32] bf16
        dxv = pm3[:, :, 1:19:2].transpose([0, 2, 1])
        sgin = pm3[:, :, 18:31].transpose([0, 2, 1])   # [p, 13, 32]
        with tc.tile_pool(name="ph_e", bufs=1) as pe:
            def t288(tag, dt=F32):
                return pe.tile([P, K * 32], dt, tag=tag, name=tag)

            py = t288("py"); px = t288("px")
            iy = t288("iy"); ix = t288("ix")
            wy = t288("wy"); wx = t288("wx")
            u = t288("u"); vv = t288("vv")
            a = t288("a"); bw = t288("bw")
            m = t288("m")
            idxf = t288("idxf")
            sg = pe.tile([P, 13 * 32], F32, tag="sg", name="sg")

            v3 = lambda t: _r3(t, 32)  # [128, 9, 32]

            # py = dy + ybase ; px = dx + xbase  (bf16 in, f32 out)
            nc.vector.tensor_tensor(v3(py), dyv, v3(yb), op=OP.add)
            nc.vector.tensor_tensor(v3(px), dxv, v3(xb), op=OP.add)
            for t in (py, px):
                nc.vector.tensor_scalar(
                    t[:, :], t[:, :], 66.4, 0.6, op0=OP.min, op1=OP.max
                )
            # floor via round-to-nearest(v - 0.5)
            nc.vector.tensor_scalar(iy[:, :], py[:, :], 0.5, MAGIC, op0=OP.subtract, op1=OP.add)
            nc.vector.tensor_scalar(iy[:, :], iy[:, :], MAGIC, None, op0=OP.subtract)
            nc.vector.tensor_scalar(ix[:, :], px[:, :], 0.5, MAGIC, op0=OP.subtract, op1=OP.add)
            nc.vector.tensor_scalar(ix[:, :], ix[:, :], MAGIC, None, op0=OP.subtract)
            nc.vector.tensor_tensor(wy[:, :], py[:, :], iy[:, :], op=OP.subtract)
            nc.vector.tensor_tensor(wx[:, :], px[:, :], ix[:, :], op=OP.subtract)
            # gather index = iy*68 + ix
            nc.vector.tensor_scalar(idxf[:, :], iy[:, :], 68.0, None, op0=OP.mult)
            nc.vector.tensor_tensor(idxf[:, :], idxf[:, :], ix[:, :], op=OP.add)
            nc.vector.tensor_copy(idx32[:, :], idxf[:, :])

            # mask: sigmoid(std_mod) * sigmoid(corner sel; absent taps -> 0.5)
            nc.scalar.activation(_r3(sg, 32), sgin, AF.Sigmoid)
            sgr = _r3(sg, 32)  # [128, 13, 32]
            for ci, k in enumerate((0, 2, 6, 8)):
                nc.vector.tensor_tensor(
                    m[:, k * 32 : (k + 1) * 32], sgr[:, k, :], sgr[:, 9 + ci, :],
                    op=OP.mult,
                )
            for k in (1, 3, 4, 5, 7):
                nc.vector.tensor_scalar(
                    m[:, k * 32 : (k + 1) * 32], sgr[:, k, :], 0.5, None, op0=OP.mult
                )

            # corner scales (mask folded): s_cr = m * wy_part * wx_part,
            # written into sall corner-interleaved + pair-duplicated.
            nc.vector.tensor_scalar(u[:, :], wy[:, :], -1.0, 1.0, op0=OP.mult, op1=OP.add)
            nc.vector.tensor_scalar(vv[:, :], wx[:, :], -1.0, 1.0, op0=OP.mult, op1=OP.add)
            nc.vector.tensor_tensor(a[:, :], m[:, :], u[:, :], op=OP.mult)    # (1-wy)*m
            nc.vector.tensor_tensor(bw[:, :], m[:, :], wy[:, :], op=OP.mult)  # wy*m
            sall8 = sall.rearrange("p (kb e) -> p kb e", e=8)  # [128, 288, 8]
            pair = lambda t: t.rearrange("p (n o) -> p n o", o=1).broadcast_to(
                (P, K * 32, 2)
            )
            # corner order = gathered patch layout [v00 | v10 | v01 | v11]
            nc.vector.tensor_tensor(sall8[:, :, 0:2], pair(a), pair(vv), op=OP.mult)
            nc.vector.tensor_tensor(sall8[:, :, 2:4], pair(bw), pair(vv), op=OP.mult)
            nc.vector.tensor_tensor(sall8[:, :, 4:6], pair(a), pair(wx), op=OP.mult)
            nc.vector.tensor_tensor(sall8[:, :, 6:8], pair(bw), pair(wx), op=OP.mult)

        # ============== phase C2: std conv (overlaps gather ramp) =======
        for n in range(8):
            ps = ps_conv.tile([P, 512], F32, tag="ps_c", name="ps_c")
            for k in range(K):
                nc.tensor.matmul(
                    ps[:, :], wstd[:, k * P : (k + 1) * P],
                    conv_rhs(n, k // 3, k % 3),
                    start=(k == 0), stop=(k == K - 1),
                )
            nc.scalar.activation(xstd[:, n * 512 : (n + 1) * 512], ps[:, :], AF.Copy)

        # ========== phase G: gather + combine + def conv + fuse =========
        with tc.tile_pool(name="gpool", bufs=4) as gpool, \
             tc.tile_pool(name="qpool", bufs=2) as qpool, \
             tc.tile_pool(name="spool", bufs=2) as spool, \
             tc.tile_pool(name="tpool", bufs=2) as tpool, \
             tc.tile_pool(name="ph_h", bufs=2) as ph, \
             tc.tile_pool(name="ps_fus", bufs=2, space="PSUM") as ps_fus, \
             tc.tile_pool(name="ps_def", bufs=1, space="PSUM") as ps_def:
            for w in range(NW):
                psd = ps_def.tile([P, WBLK * P], F32, tag="psd", name="psd")
                for k in range(K):
                    c0 = k * 32 + w * WBLK
                    g = gpool.tile([P, WBLK, 512], BF16, tag="g", name="g")
                    for bb in range(WBLK):
                        nc.gpsimd.indirect_dma_start(
                            out=g[:, bb, :],
                            out_offset=None,
                            in_=x2_d[:, :],
                            in_offset=bass.IndirectOffsetOnAxis(
                                ap=idx32[:, c0 + bb : c0 + bb + 1], axis=0
                            ),
                        )
                    # combine: q = g * scales (2x), then 4:1 tree add (2x)
                    q = qpool.tile([P, WBLK * 512], BF16, tag="q", name="q")
                    t = qpool.tile([P, WBLK * 256], BF16, tag="t", name="t")
                    samp = spool.tile([P, WBLK * P], BF16, tag="samp", name="samp")
                    gv = g[:, :, :].rearrange(
                        "p b (c x e) -> p (b c) x e", c=4, e=2
                    )
                    sv = (
                        sall[:, c0 * 8 : (c0 + WBLK) * 8]
                        .rearrange("p (bc o e) -> p bc o e", o=1, e=2)
                        .broadcast_to((P, WBLK * 4, 64, 2))
                    )
                    qv = q.rearrange("p (bc x e) -> p bc x e", bc=WBLK * 4, e=2)
                    nc.vector.tensor_tensor(qv, gv, sv, op=OP.mult)
                    qh = q.rearrange("p (b h) -> p b h", h=512)
                    th = t.rearrange("p (b h) -> p b h", h=256)
                    nc.vector.tensor_tensor(th, qh[:, :, 0:256], qh[:, :, 256:512], op=OP.add)
                    sh = samp.rearrange("p (b h) -> p b h", h=128)
                    nc.vector.tensor_tensor(sh, th[:, :, 0:128], th[:, :, 128:256], op=OP.add)

                    # rhsT[ch, bb, pos] = samp[pos, bb*128+ch] (one XBAR transpose)
                    rhsT = tpool.tile([P, WBLK, P], BF16, tag="rhsT", name="rhsT")
                    nc.scalar.dma_start(rhsT[:, :, :], samp[:, :], transpose=True)
                    for bb in range(WBLK):
                        # start marks the whole 2KB PSUM bank (4 blocks)
                        # pending-zero, so only the first block of each bank
                        # may set it.
                        nc.tensor.matmul(
                            psd[:, bb * P : (bb + 1) * P],
                            wdef[:, k * P : (k + 1) * P],
                            rhsT[:, bb, :],
                            start=(k == 0 and bb % 4 == 0),
                            stop=(k == K - 1 and bb % 4 == 3),
                            skip_group_check=True,
                        )
                nc.scalar.activation(
                    xdef[:, w * WBLK * P : (w + 1) * WBLK * P], psd[:, :], AF.Copy
                )
                # ====== fused 1x1 conv for this wave's 4 n-tiles ========
                for n in range(w * 4, w * 4 + 4):
                    ps = ps_fus.tile([P, 512], F32, tag="ps_h", name="ps_h")
                    nc.tensor.matmul(
                        ps[:, :], wfus[:, 0:P], xstd[:, n * 512 : (n + 1) * 512],
                        start=True, stop=False,
                    )
                    nc.tensor.matmul(
                        ps[:, :], wfus[:, P : 2 * P], xdef[:, n * 512 : (n + 1) * 512],
                        start=False, stop=True,
                    )
                    stage = ph.tile([P, 512], F32, tag="stage", name="stage")
                    nc.scalar.activation(stage[:, :], ps[:, :], AF.Identity, bias=bfus[:, :])
                    nc.sync.dma_start(out_d[:, n * 512 : (n + 1) * 512], stage[:, :])

    return nc


def _consts(W_std, b_std, W_off, b_off, W_mod, b_mod, W_def, b_def, W_fus, b_fus):
    """Host-side constant prep (shared across cores)."""
    f = np.float32
    wstd = np.transpose(W_std, (2, 3, 1, 0)).reshape(K, P, P)  # [k, c, o]
    wom_full = np.concatenate([W_off, W_mod], axis=0)  # [31, 128, 3, 3]
    wom = np.zeros((K, P, 32), f)
    wom[:, :, :31] = np.transpose(wom_full, (2, 3, 1, 0)).reshape(K, P, 31)
    wdef = np.transpose(W_def, (2, 3, 1, 0)).reshape(K, P, P)
    wf = W_fus[:, :, 0, 0]  # [128, 256]
    wfus = np.stack([wf[:, :P].T, wf[:, P:].T], axis=0)  # [2, c, o]
    bfus = (b_fus + wf[:, :P] @ b_std + wf[:, P:] @ b_def).reshape(P, 1)
    bom = np.zeros((32, 1), f)
    bom[:18, 0] = b_off
    bom[18:31, 0] = b_mod
    # ybase/xbase in [p, k*32+b] layout: j = b*128 + p
    pp, kk, bb2 = np.meshgrid(np.arange(P), np.arange(K), np.arange(32), indexing="ij")
    j = bb2 * 128 + pp
    yb = ((j >> 6) + (kk // 3) + 1).astype(f).reshape(P, K * 32)
    xb = ((j & 63) + (kk % 3) + 1).astype(f).reshape(P, K * 32)
    return dict(
        wstd=wstd.astype(NPBF), wom=wom.astype(NPBF), wdef=wdef.astype(NPBF),
        wfus=wfus.astype(NPBF), bfus=bfus.astype(f), bom=bom.astype(f),
        yb=yb, xb=xb,
    )


_NC_CACHE = {}


def _get_nc():
    if "nc" not in _NC_CACHE:
        nc = build_nc()
        nc.finalize()
        _NC_CACHE["nc"] = nc
    return _NC_CACHE["nc"]


def kernel(x, W_std, b_std, W_off, b_off, W_corner, b_corner, W_mod, b_mod,
           W_def, b_def, W_fus, b_fus, **kw):
    consts = _consts(
        np.asarray(W_std, np.float32), np.asarray(b_std, np.float32),
        np.asarray(W_off, np.float32), np.asarray(b_off, np.float32),
        np.asarray(W_mod, np.float32), np.asarray(b_mod, np.float32),
        np.asarray(W_def, np.float32), np.asarray(b_def, np.float32),
        np.asarray(W_fus, np.float32), np.asarray(b_fus, np.float32),
    )
    x = np.asarray(x, np.float32)
    B = x.shape[0]
    assert B == N_CORES, x.shape
    in_maps = []
    for b in range(B):
        im = dict(consts)
        im["x"] = np.ascontiguousarray(x[b].reshape(P, NP))
        in_maps.append(im)
    nc = _get_nc()
    res = run_bass_kernel_spmd(nc, in_maps, core_ids=list(range(N_CORES)))
    out = np.stack([r["out"].reshape(P, H, H) for r in res.results], axis=0)
    return out.astype(np.float32)


if __name__ == "__main__":
    nc = build_nc()
    nc.finalize()
    print("built ok")


# revision 6
# speedup vs baseline: 1.0450x; 1.0450x over previous
"""Trainium2 Bass kernel for nn_DABConv (deformable attention-ish conv).

Data-parallel over batch: 8 samples -> 8 NeuronCores, one sample per core.

v3: gather-stream-bound design (288 indirect DMAs x ~1.1us serialized on
GpSimd SWDGE desc-gen).  Everything else hides under that stream:
  - x loaded f32 via sync DMA, DVE cast into zero-padded bf16 xp.
  - channel-last image xcl built with ONE DMA-transpose (XBAR), not PE.
  - offset/modulator conv computed in TWO halves (wave-granular) so wave-0
    index math finishes early and the gather stream starts ~40us in; each
    half accumulates into one big PSUM tile -> ONE copy -> ONE DMA-transpose.
  - per (wave,tap): 16x 1KB-patch indirect gathers -> 3 DVE combine ops ->
    ONE DMA-transpose (samp -> rhsT, alternating sync/scalar queues) -> 16
    accumulating def-conv matmuls.
  - PSUM->SBUF copies run on DVE (vector) to keep the scalar queue free for
    the rhsT transposes.
  - fused 1x1 conv + output DMA emitted per wave; the last tap of wave 1 is
    processed in two 8-block chunks to shorten the tail.

Patch gather trick: x2cl[(r,c)] = [x_cl[r,c] | x_cl[r+1,c]] bf16 in DRAM, so
one 1KB gathered element at row iy*68+ix = the full 2x2 bilinear patch
[v00|v10|v01|v11] x 128ch.
"""

import numpy as np
import ml_dtypes
from contextlib import ExitStack

import concourse.bass as bass
import concourse.bacc as bacc
import concourse.mybir as mybir
from concourse.tile import TileContext
from concourse.bass_utils import run_bass_kernel_spmd

AF = mybir.ActivationFunctionType
OP = mybir.AluOpType
F32 = mybir.dt.float32
BF16 = mybir.dt.bfloat16
NPBF = ml_dtypes.bfloat16

P = 128
H = 64
HP = 68          # padded image side (pad=2 each side)
NP = H * H       # 4096 output positions
NPAD = HP * HP   # 4624 padded positions
NBLK = 37        # ceil(4624/128)
XPW = NBLK * P   # 4736 (xp free width incl. zero tail for the transpose)
K = 9
MAGIC = 12582912.0  # 2**23 + 2**22: float32 round-to-int trick
NW = 2           # waves over position blocks (PSUM capacity)
WBLK = 16        # 128-position blocks per wave
N_CORES = 8


def _r3(ap, inner):
    return ap.rearrange("p (a b) -> p a b", b=inner)


def build_nc():
    nc = bacc.Bacc("TRN2", target_bir_lowering=False, debug=False)

    x_d = nc.dram_tensor("x", [P, NP], F32, kind="ExternalInput")
    wstd_d = nc.dram_tensor("wstd", [K, P, P], BF16, kind="ExternalInput")
    wom_d = nc.dram_tensor("wom", [K, P, 32], BF16, kind="ExternalInput")
    wdef_d = nc.dram_tensor("wdef", [K, P, P], BF16, kind="ExternalInput")
    wfus_d = nc.dram_tensor("wfus", [2, P, P], BF16, kind="ExternalInput")
    bfus_d = nc.dram_tensor("bfus", [P, 1], F32, kind="ExternalInput")
    bom_d = nc.dram_tensor("bom", [32, 1], F32, kind="ExternalInput")
    yb_d = nc.dram_tensor("yb", [P, K * 32], F32, kind="ExternalInput")
    xb_d = nc.dram_tensor("xb", [P, K * 32], F32, kind="ExternalInput")
    out_d = nc.dram_tensor("out", [P, NP], F32, kind="ExternalOutput")
    # internal scratch: channel-last row-pair image; row j = padded pos
    # (r,c) holds [x_cl[r,c] | x_cl[r+1,c]] x 128ch bf16; a 512-elem gather
    # at row j spans rows j, j+1 = the full 2x2 bilinear patch.
    x2_d = nc.dram_tensor("x2cl", [NBLK * 128, 256], BF16)

    with TileContext(nc) as tc, ExitStack() as top:
        const = top.enter_context(tc.tile_pool(name="const", bufs=1))
        main = top.enter_context(tc.tile_pool(name="main", bufs=1))

        # ---- const loads ----
        wstd = const.tile([P, K * P], BF16, tag="wstd", name="wstd")
        nc.sync.dma_start(_r3(wstd, P), wstd_d[:, :, :].transpose([1, 0, 2]))
        wom = const.tile([P, K * 32], BF16, tag="wom", name="wom")
        nc.sync.dma_start(_r3(wom, 32), wom_d[:, :, :].transpose([1, 0, 2]))
        wdef = const.tile([P, K * P], BF16, tag="wdef", name="wdef")
        nc.sync.dma_start(_r3(wdef, P), wdef_d[:, :, :].transpose([1, 0, 2]))
        wfus = const.tile([P, 2 * P], BF16, tag="wfus", name="wfus")
        nc.sync.dma_start(_r3(wfus, P), wfus_d[:, :, :].transpose([1, 0, 2]))
        bfus = const.tile([P, 1], F32, tag="bfus", name="bfus")
        nc.sync.dma_start(bfus[:, :], bfus_d[:, :])
        bom = const.tile([32, 1], F32, tag="bom", name="bom")
        nc.sync.dma_start(bom[:, :], bom_d[:, :])
        yb = const.tile([P, K * 32], F32, tag="yb", name="yb")
        nc.sync.dma_start(yb[:, :], yb_d[:, :])
        xb = const.tile([P, K * 32], F32, tag="xb", name="xb")
        nc.sync.dma_start(xb[:, :], xb_d[:, :])

        # ---- long-lived tiles ----
        xp = main.tile([P, XPW], BF16, tag="xp", name="xp")
        xstd = main.tile([P, NP], BF16, tag="xstd", name="xstd")
        # pmT halves: [p, b(16), ch(128)], ch>=32 garbage (never read)
        pmT = [main.tile([P, WBLK * P], BF16, tag=f"pmT{h}", name=f"pmT{h}")
               for h in range(2)]
        omb = [main.tile([P, WBLK * P], BF16, tag=f"omb{h}", name=f"omb{h}")
               for h in range(2)]  # [128, 2048]; rows 0..31 valid
        idx32 = main.tile([P, K * 32], mybir.dt.int32, tag="idx32", name="idx32")
        # corner scales, corner-interleaved + pair-duplicated:
        # col = (k*32+b)*8 + corner*2 + {0,1}; corner order matches the
        # gathered patch layout [v00 | v10 | v01 | v11].
        sall = main.tile([P, K * 32 * 8], BF16, tag="sall", name="sall")
        xdef = main.tile([P, NP], BF16, tag="xdef", name="xdef")

        def conv_rhs(n, ki, kj):
            base = (8 * n + ki + 1) * HP
            v = xp[:, base : base + 8 * HP]
            return _r3(v, HP)[:, :, kj + 1 : kj + 1 + H]

        # ================= phase A: load + padded bf16 image ============
        xpv = _r3(xp[:, 0:NPAD], HP)  # [128, 68, 68]
        with tc.tile_pool(name="ph_a", bufs=1) as pa:
            x_sb = pa.tile([P, NP], F32, tag="x_sb", name="x_sb")
            nc.sync.dma_start(x_sb[:, :], x_d[:, :])
            nc.vector.memset(xp[:, 0 : 2 * HP], 0.0)
            nc.vector.memset(xp[:, 66 * HP : NPAD], 0.0)
            nc.vector.memset(xpv[:, 2:66, 0:2], 0.0)
            nc.vector.memset(xpv[:, 2:66, 66:68], 0.0)
            nc.vector.memset(xp[:, NPAD:XPW], 0.0)
            nc.vector.tensor_copy(xpv[:, 2:66, 2:66], _r3(x_sb[:, :], H))

            # ========== phase B: channel-last image + row-pair DRAM =====
            xcl = pa.tile([P, NBLK, P], BF16, tag="xcl", name="xcl")
            # xcl[j, b, i] = xp[i, b*128 + j]  (one XBAR transpose)
            nc.sync.dma_start(xcl[:, :, :], xp[:, :], transpose=True)
            x2r = x2_d[:, :].rearrange("(b p) c -> p b c", p=P)  # [128, 37, 256]
            xclr = xcl[:, :, :]
            # plane 0: entry (r,c) <- x_cl[r,c]
            nc.sync.dma_start(x2r[:, :, 0:P], xclr)
            # plane 1: entry (r,c) <- x_cl[r+1,c]  (source shifted by 68 pos)
            nc.sync.dma_start(x2r[0:60, :, P:256], xclr[68:128, :, :])
            nc.sync.dma_start(x2r[60:128, 0 : NBLK - 1, P:256], xclr[0:68, 1:NBLK, :])

        # ===== phases C/D/E per half: om conv -> transpose -> math ======
        with tc.tile_pool(name="ps_om", bufs=1, space="PSUM") as ps_om, \
             tc.tile_pool(name="ph_e", bufs=1) as pe_pool:

            def t288(tag, dt=F32):
                return pe_pool.tile([P, K * 32], dt, tag=tag, name=tag)

            py = t288("py"); px = t288("px")
            iy = t288("iy"); ix = t288("ix")
            wy = t288("wy"); wx = t288("wx")
            u = t288("u"); vv = t288("vv")
            a = t288("a"); bw = t288("bw")
            m = t288("m")
            idxf = t288("idxf")
            sg = pe_pool.tile([P, 13 * 32], F32, tag="sg", name="sg")
            sall8 = sall.rearrange("p (kb e) -> p kb e", e=8)  # [128, 288, 8]

            for h in range(2):
                # ---- C: om conv for n-tiles of this half --------------
                ps = ps_om.tile([32, 4 * 512], F32, tag="om_ps", name="om_ps")
                for nl, n in enumerate(range(4 * h, 4 * h + 4)):
                    for k in range(K):
                        nc.tensor.matmul(
                            ps[:, nl * 512 : (nl + 1) * 512],
                            wom[:, k * 32 : (k + 1) * 32],
                            conv_rhs(n, k // 3, k % 3),
                            start=(k == 0), stop=(k == K - 1),
                        )
                nc.scalar.activation(omb[h][:32, :], ps[:, :], AF.Identity,
                                     bias=bom[:, :])
                # ---- D: one XBAR transpose to position-major ----------
                # pmT[h][j, b, ch] = omb[h][ch, b*128+j]
                nc.scalar.dma_start(_r3(pmT[h], P), omb[h][:, :], transpose=True)

                # ---- E: index & weight math for this half's 16 blocks -
                pm3 = _r3(pmT[h], P)                          # [128, 16, 128]
                dyv = pm3[:, :, 0:18:2].transpose([0, 2, 1])  # [p, 9, 16]
                dxv = pm3[:, :, 1:19:2].transpose([0, 2, 1])
                sgin = pm3[:, :, 18:31].transpose([0, 2, 1])  # [p, 13, 16]
                bs = slice(h * WBLK, (h + 1) * WBLK)
                v3 = lambda t: _r3(t, 32)[:, :, bs]  # [128, 9, 16] slice

                nc.vector.tensor_tensor(v3(py), dyv, v3(yb), op=OP.add)
                nc.vector.tensor_tensor(v3(px), dxv, v3(xb), op=OP.add)
                for t in (py, px):
                    nc.vector.tensor_scalar(
                        v3(t), v3(t), 66.4, 0.6, op0=OP.min, op1=OP.max
                    )
                # floor via round-to-nearest(v - 0.5)
                nc.vector.tensor_scalar(v3(iy), v3(py), 0.5, MAGIC, op0=OP.subtract, op1=OP.add)
                nc.vector.tensor_scalar(v3(iy), v3(iy), MAGIC, None, op0=OP.subtract)
                nc.vector.tensor_scalar(v3(ix), v3(px), 0.5, MAGIC, op0=OP.subtract, op1=OP.add)
                nc.vector.tensor_scalar(v3(ix), v3(ix), MAGIC, None, op0=OP.subtract)
                nc.vector.tensor_tensor(v3(wy), v3(py), v3(iy), op=OP.subtract)
                nc.vector.tensor_tensor(v3(wx), v3(px), v3(ix), op=OP.subtract)
                # gather index = iy*68 + ix
                nc.vector.tensor_scalar(v3(idxf), v3(iy), 68.0, None, op0=OP.mult)
                nc.vector.tensor_tensor(v3(idxf), v3(idxf), v3(ix), op=OP.add)
                nc.vector.tensor_copy(_r3(idx32, 32)[:, :, bs], v3(idxf))

                # mask: sigmoid(std_mod) * sigmoid(corner; absent -> 0.5)
                sgv = _r3(sg, 32)[:, :, bs]
                nc.scalar.activation(sgv, sgin, AF.Sigmoid)
                sgr = _r3(sg, 32)
                for ci, k in enumerate((0, 2, 6, 8)):
                    nc.vector.tensor_tensor(
                        m[:, k * 32 + h * WBLK : k * 32 + h * WBLK + WBLK],
                        sgr[:, k, bs], sgr[:, 9 + ci, bs], op=OP.mult,
                    )
                for k in (1, 3, 4, 5, 7):
                    nc.vector.tensor_scalar(
                        m[:, k * 32 + h * WBLK : k * 32 + h * WBLK + WBLK],
                        sgr[:, k, bs], 0.5, None, op0=OP.mult,
                    )

                # corner scales (mask folded) into sall, pair-duplicated
                nc.vector.tensor_scalar(v3(u), v3(wy), -1.0, 1.0, op0=OP.mult, op1=OP.add)
                nc.vector.tensor_scalar(v3(vv), v3(wx), -1.0, 1.0, op0=OP.mult, op1=OP.add)
                nc.vector.tensor_tensor(v3(a), v3(m), v3(u), op=OP.mult)
                nc.vector.tensor_tensor(v3(bw), v3(m), v3(wy), op=OP.mult)
                s8 = sall8.rearrange("p (k b) e -> p k b e", k=K)[:, :, bs, :]
                pair = lambda t: _r3(t, 32)[:, :, bs].unsqueeze(3).broadcast_to(
                    (P, K, WBLK, 2))
                # corner order = patch layout [v00 | v10 | v01 | v11]
                nc.vector.tensor_tensor(s8[:, :, :, 0:2], pair(a), pair(vv), op=OP.mult)
                nc.vector.tensor_tensor(s8[:, :, :, 2:4], pair(bw), pair(vv), op=OP.mult)
                nc.vector.tensor_tensor(s8[:, :, :, 4:6], pair(a), pair(wx), op=OP.mult)
                nc.vector.tensor_tensor(s8[:, :, :, 6:8], pair(bw), pair(wx), op=OP.mult)

        # ============== std conv (overlaps gather ramp) =================
        with tc.tile_pool(name="ps_std", bufs=1, space="PSUM") as ps_std:
            for hh in range(2):
                pss = ps_std.tile([P, 2048], F32, tag="std_ps", name="std_ps")
                for nl, n in enumerate(range(4 * hh, 4 * hh + 4)):
                    for k in range(K):
                        nc.tensor.matmul(
                            pss[:, nl * 512 : (nl + 1) * 512],
                            wstd[:, k * P : (k + 1) * P],
                            conv_rhs(n, k // 3, k % 3),
                            start=(k == 0), stop=(k == K - 1),
                        )
                nc.vector.tensor_copy(
                    xstd[:, hh * 2048 : (hh + 1) * 2048], pss[:, :])

        # ========== phase G: gather + combine + def conv + fuse =========
        with tc.tile_pool(name="gpool", bufs=4) as gpool, \
             tc.tile_pool(name="qpool", bufs=2) as qpool, \
             tc.tile_pool(name="tpool", bufs=1) as tp1, \
             tc.tile_pool(name="spool", bufs=3) as spool, \
             tc.tile_pool(name="rpool", bufs=3) as rpool, \
             tc.tile_pool(name="ph_h", bufs=2) as ph, \
             tc.tile_pool(name="ps_fus", bufs=2, space="PSUM") as ps_fus, \
             tc.tile_pool(name="ps_def", bufs=1, space="PSUM", side="right") as ps_def:

            def chunk(w, k, cb0, ncb, psd):
                """Process blocks [cb0, cb0+ncb) of wave w, tap k."""
                c0 = k * 32 + w * WBLK + cb0
                g = gpool.tile([P, WBLK, 512], BF16, tag="g", name="g")
                g = g[:, 0:ncb, :]
                for bb in range(ncb):
                    nc.gpsimd.indirect_dma_start(
                        out=g[:, bb, :],
                        out_offset=None,
                        in_=x2_d[:, :],
                        in_offset=bass.IndirectOffsetOnAxis(
                            ap=idx32[:, c0 + bb : c0 + bb + 1], axis=0
                        ),
                    )
                # combine: q = g * scales (2x), then 4:1 tree add (2x)
                q = qpool.tile([P, WBLK * 512], BF16, tag="q", name="q")[:, 0 : ncb * 512]
                t = tp1.tile([P, WBLK * 256], BF16, tag="t", name="t")[:, 0 : ncb * 256]
                samp = spool.tile([P, WBLK * P], BF16, tag="samp", name="samp")[:, 0 : ncb * P]
                gv = g[:, :, :].rearrange("p b (c x e) -> p (b c) x e", c=4, e=2)
                sv = (
                    sall[:, c0 * 8 : (c0 + ncb) * 8]
                    .rearrange("p (bc o e) -> p bc o e", o=1, e=2)
                    .broadcast_to((P, ncb * 4, 64, 2))
                )
                qv = q.rearrange("p (bc x e) -> p bc x e", bc=ncb * 4, e=2)
                nc.vector.tensor_tensor(qv, gv, sv, op=OP.mult)
                qh = q.rearrange("p (b h) -> p b h", h=512)
                th = t.rearrange("p (b h) -> p b h", h=256)
                nc.vector.tensor_tensor(th, qh[:, :, 0:256], qh[:, :, 256:512], op=OP.add)
                sh = samp.rearrange("p (b h) -> p b h", h=128)
                nc.vector.tensor_tensor(sh, th[:, :, 0:128], th[:, :, 128:256], op=OP.add)

                # rhsT[ch, bb, pos] = samp[pos, bb*128+ch] (one XBAR transpose)
                rhsT = rpool.tile([P, WBLK, P], BF16, tag="rhsT", name="rhsT")
                rhsT = rhsT[:, 0:ncb, :]
                eng = nc.sync if (k % 2 == 0) else nc.scalar
                eng.dma_start(rhsT[:, :, :], samp[:, :], transpose=True)
                for bb in range(ncb):
                    gb = cb0 + bb
                    # start marks the whole 2KB PSUM bank (4 blocks)
                    # pending-zero, so only the first block of each bank
                    # may set it.
                    nc.tensor.matmul(
                        psd[:, gb * P : (gb + 1) * P],
                        wdef[:, k * P : (k + 1) * P],
                        rhsT[:, bb, :],
                        start=(k == 0 and gb % 4 == 0),
                        stop=(k == K - 1 and gb % 4 == 3),
                        skip_group_check=True,
                    )

            for w in range(NW):
                psd = ps_def.tile([P, WBLK * P], F32, tag="psd", name="psd")
                for k in range(K):
                    if w == 1 and k == K - 1:
                        chunk(w, k, 0, 8, psd)
                        chunk(w, k, 8, 8, psd)
                    else:
                        chunk(w, k, 0, WBLK, psd)
                nc.vector.tensor_copy(
                    xdef[:, w * WBLK * P : (w + 1) * WBLK * P], psd[:, :])
                # ====== fused 1x1 conv for this wave's 4 n-tiles ========
                for n in range(w * 4, w * 4 + 4):
                    ps = ps_fus.tile([P, 512], F32, tag="ps_h", name="ps_h")
                    nc.tensor.matmul(
                        ps[:, :], wfus[:, 0:P], xstd[:, n * 512 : (n + 1) * 512],
                        start=True, stop=False,
                    )
                    nc.tensor.matmul(
                        ps[:, :], wfus[:, P : 2 * P], xdef[:, n * 512 : (n + 1) * 512],
                        start=False, stop=True,
                    )
                    stage = ph.tile([P, 512], F32, tag="stage", name="stage")
                    nc.scalar.activation(stage[:, :], ps[:, :], AF.Identity, bias=bfus[:, :])
                    nc.sync.dma_start(out_d[:, n * 512 : (n + 1) * 512], stage[:, :])

    return nc


def _consts(W_std, b_std, W_off, b_off, W_mod, b_mod, W_def, b_def, W_fus, b_fus):
    """Host-side constant prep (shared across cores)."""
    f = np.float32
    wstd = np.transpose(W_std, (2, 3, 1, 0)).reshape(K, P, P)  # [k, c, o]
    wom_full = np.concatenate([W_off, W_mod], axis=0)  # [31, 128, 3, 3]
    wom = np.zeros((K, P, 32), f)
    wom[:, :, :31] = np.transpose(wom_full, (2, 3, 1, 0)).reshape(K, P, 31)
    wdef = np.transpose(W_def, (2, 3, 1, 0)).reshape(K, P, P)
    wf = W_fus[:, :, 0, 0]  # [128, 256]
    wfus = np.stack([wf[:, :P].T, wf[:, P:].T], axis=0)  # [2, c, o]
    bfus = (b_fus + wf[:, :P] @ b_std + wf[:, P:] @ b_def).reshape(P, 1)
    bom = np.zeros((32, 1), f)
    bom[:18, 0] = b_off
    bom[18:31, 0] = b_mod
    # ybase/xbase in [p, k*32+b] layout: j = b*128 + p
    pp, kk, bb2 = np.meshgrid(np.arange(P), np.arange(K), np.arange(32), indexing="ij")
    j = bb2 * 128 + pp
    yb = ((j >> 6) + (kk // 3) + 1).astype(f).reshape(P, K * 32)
    xb = ((j & 63) + (kk % 3) + 1).astype(f).reshape(P, K * 32)
    return dict(
        wstd=wstd.astype(NPBF), wom=wom.astype(NPBF), wdef=wdef.astype(NPBF),
        wfus=wfus.astype(NPBF), bfus=bfus.astype(f), bom=bom.astype(f),
        yb=yb, xb=xb,
    )


_NC_CACHE = {}


def _get_nc():
    if "nc" not in _NC_CACHE:
        nc = build_nc()
        nc.finalize()
        _NC_CACHE["nc"] = nc
    return _NC_CACHE["nc"]


def kernel(x, W_std, b_std, W_off, b_off, W_corner, b_corner, W_mod, b_mod,
           W_def, b_def, W_fus, b_fus, **kw):
    consts = _consts(
        np.asarray(W_std, np.float32), np.asarray(b_std, np.float32),
        np.asarray(W_off, np.float32), np.asarray(b_off, np.float32),
        np.asarray(W_mod, np.float32), np.asarray(b_mod, np.float32),
        np.asarray(W_def, np.float32), np.asarray(b_def, np.float32),
        np.asarray(W_fus, np.float32), np.asarray(b_fus, np.float32),
    )
    x = np.asarray(x, np.float32)
    B = x.shape[0]
    assert B == N_CORES, x.shape
    in_maps = []
    for b in range(B):
        im = dict(consts)
        im["x"] = np.ascontiguousarray(x[b].reshape(P, NP))
        in_maps.append(im)
    nc = _get_nc()
    res = run_bass_kernel_spmd(nc, in_maps, core_ids=list(range(N_CORES)))
    out = np.stack([r["out"].reshape(P, H, H) for r in res.results], axis=0)
    return out.astype(np.float32)


if __name__ == "__main__":
    nc = build_nc()
    nc.finalize()
    print("built ok")


# revision 7
# speedup vs baseline: 1.2761x; 1.2212x over previous
"""Trainium2 Bass kernel for nn_DABConv (deformable attention-ish conv).

Data-parallel over batch: 8 samples -> 8 NeuronCores, one sample per core.

v3: gather-stream-bound design (288 indirect DMAs x ~1.1us serialized on
GpSimd SWDGE desc-gen).  Everything else hides under that stream:
  - x loaded f32 via sync DMA, DVE cast into zero-padded bf16 xp.
  - channel-last image xcl built with ONE DMA-transpose (XBAR), not PE.
  - offset/modulator conv computed in TWO halves (wave-granular) so wave-0
    index math finishes early and the gather stream starts ~40us in; each
    half accumulates into one big PSUM tile -> ONE copy -> ONE DMA-transpose.
  - per (wave,tap): 16x 1KB-patch indirect gathers -> 3 DVE combine ops ->
    ONE DMA-transpose (samp -> rhsT, alternating sync/scalar queues) -> 16
    accumulating def-conv matmuls.
  - PSUM->SBUF copies run on DVE (vector) to keep the scalar queue free for
    the rhsT transposes.
  - fused 1x1 conv + output DMA emitted per wave; the last tap of wave 1 is
    processed in two 8-block chunks to shorten the tail.

Patch gather trick: x2cl[(r,c)] = [x_cl[r,c] | x_cl[r+1,c]] bf16 in DRAM, so
one 1KB gathered element at row iy*68+ix = the full 2x2 bilinear patch
[v00|v10|v01|v11] x 128ch.
"""

import numpy as np
import ml_dtypes
from contextlib import ExitStack

import concourse.bass as bass
import concourse.bacc as bacc
import concourse.mybir as mybir
from concourse.tile import TileContext
from concourse.bass_utils import run_bass_kernel_spmd

AF = mybir.ActivationFunctionType
OP = mybir.AluOpType
F32 = mybir.dt.float32
BF16 = mybir.dt.bfloat16
NPBF = ml_dtypes.bfloat16

P = 128
H = 64
HP = 68          # padded image side (pad=2 each side)
NP = H * H       # 4096 output positions
NPAD = HP * HP   # 4624 padded positions
NBLK = 37        # ceil(4624/128)
XPW = NBLK * P   # 4736 (xp free width incl. zero tail for the transpose)
K = 9
MAGIC = 12582912.0  # 2**23 + 2**22: float32 round-to-int trick
NW = 2           # waves over position blocks (PSUM capacity)
WBLK = 16        # 128-position blocks per wave
N_CORES = 8


def _r3(ap, inner):
    return ap.rearrange("p (a b) -> p a b", b=inner)


def build_nc():
    nc = bacc.Bacc("TRN2", target_bir_lowering=False, debug=False)

    x_d = nc.dram_tensor("x", [P, NP], F32, kind="ExternalInput")
    wstd_d = nc.dram_tensor("wstd", [K, P, P], BF16, kind="ExternalInput")
    wom_d = nc.dram_tensor("wom", [K, P, 32], BF16, kind="ExternalInput")
    wdef_d = nc.dram_tensor("wdef", [K, P, P], BF16, kind="ExternalInput")
    wfus_d = nc.dram_tensor("wfus", [2, P, P], BF16, kind="ExternalInput")
    bfus_d = nc.dram_tensor("bfus", [P, 1], F32, kind="ExternalInput")
    bom_d = nc.dram_tensor("bom", [32, 1], F32, kind="ExternalInput")
    yb_d = nc.dram_tensor("yb", [P, K * 32], F32, kind="ExternalInput")
    xb_d = nc.dram_tensor("xb", [P, K * 32], F32, kind="ExternalInput")
    idn_d = nc.dram_tensor("idn", [P, P], BF16, kind="ExternalInput")
    out_d = nc.dram_tensor("out", [P, NP], F32, kind="ExternalOutput")
    # internal scratch: channel-last row-pair image; row j = padded pos
    # (r,c) holds [x_cl[r,c] | x_cl[r+1,c]] x 128ch bf16; a 512-elem gather
    # at row j spans rows j, j+1 = the full 2x2 bilinear patch.
    x2_d = nc.dram_tensor("x2cl", [NBLK * 128, 256], BF16)

    with TileContext(nc) as tc, ExitStack() as top:
        const = top.enter_context(tc.tile_pool(name="const", bufs=1))
        main = top.enter_context(tc.tile_pool(name="main", bufs=1))

        # ---- const loads ----
        wstd = const.tile([P, K * P], BF16, tag="wstd", name="wstd")
        nc.sync.dma_start(_r3(wstd, P), wstd_d[:, :, :].transpose([1, 0, 2]))
        wom = const.tile([P, K * 32], BF16, tag="wom", name="wom")
        nc.sync.dma_start(_r3(wom, 32), wom_d[:, :, :].transpose([1, 0, 2]))
        wdef = const.tile([P, K * P], BF16, tag="wdef", name="wdef")
        nc.sync.dma_start(_r3(wdef, P), wdef_d[:, :, :].transpose([1, 0, 2]))
        wfus = const.tile([P, 2 * P], BF16, tag="wfus", name="wfus")
        nc.sync.dma_start(_r3(wfus, P), wfus_d[:, :, :].transpose([1, 0, 2]))
        bfus = const.tile([P, 1], F32, tag="bfus", name="bfus")
        nc.sync.dma_start(bfus[:, :], bfus_d[:, :])
        bom = const.tile([32, 1], F32, tag="bom", name="bom")
        nc.sync.dma_start(bom[:, :], bom_d[:, :])
        yb = const.tile([P, K * 32], F32, tag="yb", name="yb")
        nc.sync.dma_start(yb[:, :], yb_d[:, :])
        xb = const.tile([P, K * 32], F32, tag="xb", name="xb")
        nc.sync.dma_start(xb[:, :], xb_d[:, :])
        idn = const.tile([P, P], BF16, tag="idn", name="idn")
        nc.sync.dma_start(idn[:, :], idn_d[:, :])

        # ---- long-lived tiles ----
        xp = main.tile([P, XPW], BF16, tag="xp", name="xp")
        xstd = main.tile([P, NP], BF16, tag="xstd", name="xstd")
        # pmT halves: [p, b(16), ch(128)], ch>=32 garbage (never read)
        pmT = [main.tile([P, WBLK * P], BF16, tag=f"pmT{h}", name=f"pmT{h}")
               for h in range(2)]
        omb = [main.tile([P, WBLK * P], BF16, tag=f"omb{h}", name=f"omb{h}")
               for h in range(2)]  # [128, 2048]; rows 0..31 valid
        idx32 = main.tile([P, K * 32], mybir.dt.int32, tag="idx32", name="idx32")
        # corner scales, corner-interleaved + pair-duplicated:
        # col = (k*32+b)*8 + corner*2 + {0,1}; corner order matches the
        # gathered patch layout [v00 | v10 | v01 | v11].
        sall = main.tile([P, K * 32 * 8], BF16, tag="sall", name="sall")
        xdef = main.tile([P, NP], BF16, tag="xdef", name="xdef")

        def conv_rhs(n, ki, kj):
            base = (8 * n + ki + 1) * HP
            v = xp[:, base : base + 8 * HP]
            return _r3(v, HP)[:, :, kj + 1 : kj + 1 + H]

        # ================= phase A: load + padded bf16 image ============
        xpv = _r3(xp[:, 0:NPAD], HP)  # [128, 68, 68]
        with tc.tile_pool(name="ph_a", bufs=1) as pa:
            x_sb = pa.tile([P, NP], F32, tag="x_sb", name="x_sb")
            nc.sync.dma_start(x_sb[:, :], x_d[:, :])
            nc.vector.memset(xp[:, 0 : 2 * HP], 0.0)
            nc.vector.memset(xp[:, 66 * HP : NPAD], 0.0)
            nc.vector.memset(xpv[:, 2:66, 0:2], 0.0)
            nc.vector.memset(xpv[:, 2:66, 66:68], 0.0)
            nc.vector.memset(xp[:, NPAD:XPW], 0.0)
            nc.vector.tensor_copy(xpv[:, 2:66, 2:66], _r3(x_sb[:, :], H))

            # ========== phase B: channel-last image + row-pair DRAM =====
            xcl = pa.tile([P, NBLK, P], BF16, tag="xcl", name="xcl")
            # xcl[j, b, i] = xp[i, b*128 + j]  (one XBAR transpose)
            nc.sync.dma_start(xcl[:, :, :], xp[:, :], transpose=True)
            x2r = x2_d[:, :].rearrange("(b p) c -> p b c", p=P)  # [128, 37, 256]
            xclr = xcl[:, :, :]
            # plane 0: entry (r,c) <- x_cl[r,c]
            nc.sync.dma_start(x2r[:, :, 0:P], xclr)
            # plane 1: entry (r,c) <- x_cl[r+1,c]  (source shifted by 68 pos)
            nc.sync.dma_start(x2r[0:60, :, P:256], xclr[68:128, :, :])
            nc.sync.dma_start(x2r[60:128, 0 : NBLK - 1, P:256], xclr[0:68, 1:NBLK, :])

        # ===== phases C/D/E per half: om conv -> transpose -> math ======
        with tc.tile_pool(name="ps_om", bufs=1, space="PSUM") as ps_om, \
             tc.tile_pool(name="ph_e", bufs=1) as pe_pool:

            def t288(tag, dt=F32):
                return pe_pool.tile([P, K * 32], dt, tag=tag, name=tag)

            py = t288("py"); px = t288("px")
            iy = t288("iy"); ix = t288("ix")
            wy = t288("wy"); wx = t288("wx")
            u = t288("u"); vv = t288("vv")
            a = t288("a"); bw = t288("bw")
            m = t288("m")
            idxf = t288("idxf")
            sg = pe_pool.tile([P, 13 * 32], F32, tag="sg", name="sg")
            sall8 = sall.rearrange("p (kb e) -> p kb e", e=8)  # [128, 288, 8]

            for h in range(2):
                # ---- C: om conv for n-tiles of this half --------------
                ps = ps_om.tile([32, 4 * 512], F32, tag="om_ps", name="om_ps")
                for nl, n in enumerate(range(4 * h, 4 * h + 4)):
                    for k in range(K):
                        nc.tensor.matmul(
                            ps[:, nl * 512 : (nl + 1) * 512],
                            wom[:, k * 32 : (k + 1) * 32],
                            conv_rhs(n, k // 3, k % 3),
                            start=(k == 0), stop=(k == K - 1),
                        )
                nc.scalar.activation(omb[h][:32, :], ps[:, :], AF.Identity,
                                     bias=bom[:, :])
                # ---- D: one XBAR transpose to position-major ----------
                # pmT[h][j, b, ch] = omb[h][ch, b*128+j]
                nc.scalar.dma_start(_r3(pmT[h], P), omb[h][:, :], transpose=True)

                # ---- E: index & weight math for this half's 16 blocks -
                pm3 = _r3(pmT[h], P)                          # [128, 16, 128]
                dyv = pm3[:, :, 0:18:2].transpose([0, 2, 1])  # [p, 9, 16]
                dxv = pm3[:, :, 1:19:2].transpose([0, 2, 1])
                sgin = pm3[:, :, 18:31].transpose([0, 2, 1])  # [p, 13, 16]
                bs = slice(h * WBLK, (h + 1) * WBLK)
                v3 = lambda t: _r3(t, 32)[:, :, bs]  # [128, 9, 16] slice

                nc.vector.tensor_tensor(v3(py), dyv, v3(yb), op=OP.add)
                nc.vector.tensor_tensor(v3(px), dxv, v3(xb), op=OP.add)
                for t in (py, px):
                    nc.vector.tensor_scalar(
                        v3(t), v3(t), 66.4, 0.6, op0=OP.min, op1=OP.max
                    )
                # floor via round-to-nearest(v - 0.5)
                nc.vector.tensor_scalar(v3(iy), v3(py), 0.5, MAGIC, op0=OP.subtract, op1=OP.add)
                nc.vector.tensor_scalar(v3(iy), v3(iy), MAGIC, None, op0=OP.subtract)
                nc.vector.tensor_scalar(v3(ix), v3(px), 0.5, MAGIC, op0=OP.subtract, op1=OP.add)
                nc.vector.tensor_scalar(v3(ix), v3(ix), MAGIC, None, op0=OP.subtract)
                nc.vector.tensor_tensor(v3(wy), v3(py), v3(iy), op=OP.subtract)
                nc.vector.tensor_tensor(v3(wx), v3(px), v3(ix), op=OP.subtract)
                # gather index = iy*68 + ix
                nc.vector.tensor_scalar(v3(idxf), v3(iy), 68.0, None, op0=OP.mult)
                nc.vector.tensor_tensor(v3(idxf), v3(idxf), v3(ix), op=OP.add)
                nc.vector.tensor_copy(_r3(idx32, 32)[:, :, bs], v3(idxf))

                # mask: sigmoid(std_mod) * sigmoid(corner; absent -> 0.5)
                sgv = _r3(sg, 32)[:, :, bs]
                nc.scalar.activation(sgv, sgin, AF.Sigmoid)
                sgr = _r3(sg, 32)
                for ci, k in enumerate((0, 2, 6, 8)):
                    nc.vector.tensor_tensor(
                        m[:, k * 32 + h * WBLK : k * 32 + h * WBLK + WBLK],
                        sgr[:, k, bs], sgr[:, 9 + ci, bs], op=OP.mult,
                    )
                for k in (1, 3, 4, 5, 7):
                    nc.vector.tensor_scalar(
                        m[:, k * 32 + h * WBLK : k * 32 + h * WBLK + WBLK],
                        sgr[:, k, bs], 0.5, None, op0=OP.mult,
                    )

                # corner scales (mask folded) into sall, pair-duplicated
                nc.vector.tensor_scalar(v3(u), v3(wy), -1.0, 1.0, op0=OP.mult, op1=OP.add)
                nc.vector.tensor_scalar(v3(vv), v3(wx), -1.0, 1.0, op0=OP.mult, op1=OP.add)
                nc.vector.tensor_tensor(v3(a), v3(m), v3(u), op=OP.mult)
                nc.vector.tensor_tensor(v3(bw), v3(m), v3(wy), op=OP.mult)
                s8 = sall8.rearrange("p (k b) e -> p k b e", k=K)[:, :, bs, :]
                pair = lambda t: _r3(t, 32)[:, :, bs].unsqueeze(3).broadcast_to(
                    (P, K, WBLK, 2))
                # corner order = patch layout [v00 | v10 | v01 | v11]
                nc.vector.tensor_tensor(s8[:, :, :, 0:2], pair(a), pair(vv), op=OP.mult)
                nc.vector.tensor_tensor(s8[:, :, :, 2:4], pair(bw), pair(vv), op=OP.mult)
                nc.vector.tensor_tensor(s8[:, :, :, 4:6], pair(a), pair(wx), op=OP.mult)
                nc.vector.tensor_tensor(s8[:, :, :, 6:8], pair(bw), pair(wx), op=OP.mult)

        # ============== std conv (overlaps gather ramp) =================
        with tc.tile_pool(name="ps_std", bufs=1, space="PSUM") as ps_std:
            for hh in range(2):
                pss = ps_std.tile([P, 2048], F32, tag="std_ps", name="std_ps")
                for nl, n in enumerate(range(4 * hh, 4 * hh + 4)):
                    for k in range(K):
                        nc.tensor.matmul(
                            pss[:, nl * 512 : (nl + 1) * 512],
                            wstd[:, k * P : (k + 1) * P],
                            conv_rhs(n, k // 3, k % 3),
                            start=(k == 0), stop=(k == K - 1),
                        )
                nc.vector.tensor_copy(
                    xstd[:, hh * 2048 : (hh + 1) * 2048], pss[:, :])

        # ========== phase G: gather + combine + def conv + fuse =========
        with tc.tile_pool(name="gpool", bufs=4) as gpool, \
             tc.tile_pool(name="qpool", bufs=2) as qpool, \
             tc.tile_pool(name="tpool", bufs=1) as tp1, \
             tc.tile_pool(name="spool", bufs=3) as spool, \
             tc.tile_pool(name="rpool", bufs=3) as rpool, \
             tc.tile_pool(name="ph_h", bufs=2) as ph, \
             tc.tile_pool(name="ps_fus", bufs=2, space="PSUM") as ps_fus, \
             tc.tile_pool(name="ps_tr", bufs=2, space="PSUM") as ps_tr, \
             tc.tile_pool(name="ps_def", bufs=1, space="PSUM", side="right") as ps_def:

            def chunk(w, k, cb0, ncb, psd):
                """Process blocks [cb0, cb0+ncb) of wave w, tap k."""
                c0 = k * 32 + w * WBLK + cb0
                g = gpool.tile([P, WBLK, 512], BF16, tag="g", name="g")
                g = g[:, 0:ncb, :]
                for bb in range(ncb):
                    nc.gpsimd.indirect_dma_start(
                        out=g[:, bb, :],
                        out_offset=None,
                        in_=x2_d[:, :],
                        in_offset=bass.IndirectOffsetOnAxis(
                            ap=idx32[:, c0 + bb : c0 + bb + 1], axis=0
                        ),
                    )
                # combine: q = g * scales (2x), then 4:1 tree add (2x)
                q = qpool.tile([P, WBLK * 512], BF16, tag="q", name="q")[:, 0 : ncb * 512]
                t = tp1.tile([P, WBLK * 256], BF16, tag="t", name="t")[:, 0 : ncb * 256]
                samp = spool.tile([P, WBLK * P], BF16, tag="samp", name="samp")[:, 0 : ncb * P]
                gv = g[:, :, :].rearrange("p b (c x e) -> p (b c) x e", c=4, e=2)
                sv = (
                    sall[:, c0 * 8 : (c0 + ncb) * 8]
                    .rearrange("p (bc o e) -> p bc o e", o=1, e=2)
                    .broadcast_to((P, ncb * 4, 64, 2))
                )
                qv = q.rearrange("p (bc x e) -> p bc x e", bc=ncb * 4, e=2)
                nc.vector.tensor_tensor(qv, gv, sv, op=OP.mult)
                qh = q.rearrange("p (b h) -> p b h", h=512)
                th = t.rearrange("p (b h) -> p b h", h=256)
                nc.vector.tensor_tensor(th, qh[:, :, 0:256], qh[:, :, 256:512], op=OP.add)
                sh = samp.rearrange("p (b h) -> p b h", h=128)
                nc.vector.tensor_tensor(sh, th[:, :, 0:128], th[:, :, 128:256], op=OP.add)

                # rhsT[ch, bb*128+pos] = samp[pos, bb*128+ch] (PE transposes)
                rhsT = rpool.tile([P, WBLK, P], BF16, tag="rhsT", name="rhsT")
                rhsT = rhsT[:, 0:ncb, :]
                for bb in range(ncb):
                    tp = ps_tr.tile([P, 256], BF16, tag="tp", name="tp")
                    nc.tensor.transpose(
                        tp[:, :P], samp[:, bb * P : (bb + 1) * P], idn
                    )
                    nc.scalar.activation(rhsT[:, bb, :], tp[:, :P], AF.Copy)
                for bb in range(ncb):
                    gb = cb0 + bb
                    # start marks the whole 2KB PSUM bank (4 blocks)
                    # pending-zero, so only the first block of each bank
                    # may set it.
                    nc.tensor.matmul(
                        psd[:, gb * P : (gb + 1) * P],
                        wdef[:, k * P : (k + 1) * P],
                        rhsT[:, bb, :],
                        start=(k == 0 and gb % 4 == 0),
                        stop=(k == K - 1 and gb % 4 == 3),
                        skip_group_check=True,
                    )

            for w in range(NW):
                psd = ps_def.tile([P, WBLK * P], F32, tag="psd", name="psd")
                for k in range(K):
                    chunk(w, k, 0, 8, psd)
                    chunk(w, k, 8, 8, psd)
                nc.vector.tensor_copy(
                    xdef[:, w * WBLK * P : (w + 1) * WBLK * P], psd[:, :])
                # ====== fused 1x1 conv for this wave's 4 n-tiles ========
                for n in range(w * 4, w * 4 + 4):
                    ps = ps_fus.tile([P, 512], F32, tag="ps_h", name="ps_h")
                    nc.tensor.matmul(
                        ps[:, :], wfus[:, 0:P], xstd[:, n * 512 : (n + 1) * 512],
                        start=True, stop=False,
                    )
                    nc.tensor.matmul(
                        ps[:, :], wfus[:, P : 2 * P], xdef[:, n * 512 : (n + 1) * 512],
                        start=False, stop=True,
                    )
                    stage = ph.tile([P, 512], F32, tag="stage", name="stage")
                    nc.scalar.activation(stage[:, :], ps[:, :], AF.Identity, bias=bfus[:, :])
                    nc.sync.dma_start(out_d[:, n * 512 : (n + 1) * 512], stage[:, :])

    return nc


def _consts(W_std, b_std, W_off, b_off, W_mod, b_mod, W_def, b_def, W_fus, b_fus):
    """Host-side constant prep (shared across cores)."""
    f = np.float32
    wstd = np.transpose(W_std, (2, 3, 1, 0)).reshape(K, P, P)  # [k, c, o]
    wom_full = np.concatenate([W_off, W_mod], axis=0)  # [31, 128, 3, 3]
    wom = np.zeros((K, P, 32), f)
    wom[:, :, :31] = np.transpose(wom_full, (2, 3, 1, 0)).reshape(K, P, 31)
    wdef = np.transpose(W_def, (2, 3, 1, 0)).reshape(K, P, P)
    wf = W_fus[:, :, 0, 0]  # [128, 256]
    wfus = np.stack([wf[:, :P].T, wf[:, P:].T], axis=0)  # [2, c, o]
    bfus = (b_fus + wf[:, :P] @ b_std + wf[:, P:] @ b_def).reshape(P, 1)
    bom = np.zeros((32, 1), f)
    bom[:18, 0] = b_off
    bom[18:31, 0] = b_mod
    # ybase/xbase in [p, k*32+b] layout: j = b*128 + p
    pp, kk, bb2 = np.meshgrid(np.arange(P), np.arange(K), np.arange(32), indexing="ij")
    j = bb2 * 128 + pp
    yb = ((j >> 6) + (kk // 3) + 1).astype(f).reshape(P, K * 32)
    xb = ((j & 63) + (kk % 3) + 1).astype(f).reshape(P, K * 32)
    return dict(
        wstd=wstd.astype(NPBF), wom=wom.astype(NPBF), wdef=wdef.astype(NPBF),
        wfus=wfus.astype(NPBF), bfus=bfus.astype(f), bom=bom.astype(f),
        yb=yb, xb=xb, idn=np.eye(P, dtype=NPBF),
    )


_NC_CACHE = {}


def _get_nc():
    if "nc" not in _NC_CACHE:
        nc = build_nc()
        nc.finalize()
        _NC_CACHE["nc"] = nc
    return _NC_CACHE["nc"]


def kernel(x, W_std, b_std, W_off, b_off, W_corner, b_corner, W_mod, b_mod,
           W_def, b_def, W_fus, b_fus, **kw):
    consts = _consts(
        np.asarray(W_std, np.float32), np.asarray(b_std, np.float32),
        np.asarray(W_off, np.float32), np.asarray(b_off, np.float32),
        np.asarray(W_mod, np.float32), np.asarray(b_mod, np.float32),
        np.asarray(W_def, np.float32), np.asarray(b_def, np.float32),
        np.asarray(W_fus, np.float32), np.asarray(b_fus, np.float32),
    )
    x = np.asarray(x, np.float32)
    B = x.shape[0]
    assert B == N_CORES, x.shape
    in_maps = []
    for b in range(B):
        im = dict(consts)
        im["x"] = np.ascontiguousarray(x[b].reshape(P, NP))
        in_maps.append(im)
    nc = _get_nc()
    res = run_bass_kernel_spmd(nc, in_maps, core_ids=list(range(N_CORES)))
    out = np.stack([r["out"].reshape(P, H, H) for r in res.results], axis=0)
    return out.astype(np.float32)


if __name__ == "__main__":
    nc = build_nc()
    nc.finalize()
    print("built ok")


# revision 8
# speedup vs baseline: 1.3123x; 1.0284x over previous
"""Trainium2 Bass kernel for nn_DABConv (deformable attention-ish conv).

Data-parallel over batch: 8 samples -> 8 NeuronCores, one sample per core.

v3: gather-stream-bound design (288 indirect DMAs x ~1.1us serialized on
GpSimd SWDGE desc-gen).  Everything else hides under that stream:
  - x loaded f32 via sync DMA, DVE cast into zero-padded bf16 xp.
  - channel-last image xcl built with ONE DMA-transpose (XBAR), not PE.
  - offset/modulator conv computed in TWO halves (wave-granular) so wave-0
    index math finishes early and the gather stream starts ~40us in; each
    half accumulates into one big PSUM tile -> ONE copy -> ONE DMA-transpose.
  - per (wave,tap): 16x 1KB-patch indirect gathers -> 3 DVE combine ops ->
    ONE DMA-transpose (samp -> rhsT, alternating sync/scalar queues) -> 16
    accumulating def-conv matmuls.
  - PSUM->SBUF copies run on DVE (vector) to keep the scalar queue free for
    the rhsT transposes.
  - fused 1x1 conv + output DMA emitted per wave; the last tap of wave 1 is
    processed in two 8-block chunks to shorten the tail.

Patch gather trick: x2cl[(r,c)] = [x_cl[r,c] | x_cl[r+1,c]] bf16 in DRAM, so
one 1KB gathered element at row iy*68+ix = the full 2x2 bilinear patch
[v00|v10|v01|v11] x 128ch.
"""

import numpy as np
import ml_dtypes
from contextlib import ExitStack

import concourse.bass as bass
import concourse.bacc as bacc
import concourse.mybir as mybir
from concourse.tile import TileContext
from concourse.bass_utils import run_bass_kernel_spmd

AF = mybir.ActivationFunctionType
OP = mybir.AluOpType
F32 = mybir.dt.float32
BF16 = mybir.dt.bfloat16
NPBF = ml_dtypes.bfloat16

P = 128
H = 64
HP = 68          # padded image side (pad=2 each side)
NP = H * H       # 4096 output positions
NPAD = HP * HP   # 4624 padded positions
NBLK = 37        # ceil(4624/128)
XPW = NBLK * P   # 4736 (xp free width incl. zero tail for the transpose)
K = 9
MAGIC = 12582912.0  # 2**23 + 2**22: float32 round-to-int trick
NW = 2           # waves over position blocks (PSUM capacity)
WBLK = 16        # 128-position blocks per wave
N_CORES = 8


def _r3(ap, inner):
    return ap.rearrange("p (a b) -> p a b", b=inner)


def build_nc():
    nc = bacc.Bacc("TRN2", target_bir_lowering=False, debug=False)

    x_d = nc.dram_tensor("x", [P, NP], F32, kind="ExternalInput")
    wstd_d = nc.dram_tensor("wstd", [K, P, P], BF16, kind="ExternalInput")
    wom_d = nc.dram_tensor("wom", [K, P, 32], BF16, kind="ExternalInput")
    wdef_d = nc.dram_tensor("wdef", [K, P, P], BF16, kind="ExternalInput")
    wfus_d = nc.dram_tensor("wfus", [2, P, P], BF16, kind="ExternalInput")
    bfus_d = nc.dram_tensor("bfus", [P, 1], F32, kind="ExternalInput")
    yb_d = nc.dram_tensor("yb", [P, K * 32], F32, kind="ExternalInput")
    xb_d = nc.dram_tensor("xb", [P, K * 32], F32, kind="ExternalInput")
    idn_d = nc.dram_tensor("idn", [P, P], BF16, kind="ExternalInput")
    bm_d = nc.dram_tensor("bm", [P, 13], F32, kind="ExternalInput")
    out_d = nc.dram_tensor("out", [P, NP], F32, kind="ExternalOutput")
    # internal scratch: channel-last row-pair image; row j = padded pos
    # (r,c) holds [x_cl[r,c] | x_cl[r+1,c]] x 128ch bf16; a 512-elem gather
    # at row j spans rows j, j+1 = the full 2x2 bilinear patch.
    x2_d = nc.dram_tensor("x2cl", [NBLK * 128, 256], BF16)

    with TileContext(nc) as tc, ExitStack() as top:
        const = top.enter_context(tc.tile_pool(name="const", bufs=1))
        main = top.enter_context(tc.tile_pool(name="main", bufs=1))

        # ---- x load first (longest-pole input data) ----
        pa_cm = tc.tile_pool(name="ph_a", bufs=1)
        pa = pa_cm.__enter__()
        x_sb = pa.tile([P, NP], F32, tag="x_sb", name="x_sb")
        nc.sync.dma_start(x_sb[:, :], x_d[:, :])

        # ---- const loads ----
        wstd = const.tile([P, K * P], BF16, tag="wstd", name="wstd")
        nc.sync.dma_start(_r3(wstd, P), wstd_d[:, :, :].transpose([1, 0, 2]))
        wom = const.tile([P, K * 32], BF16, tag="wom", name="wom")
        nc.sync.dma_start(_r3(wom, 32), wom_d[:, :, :].transpose([1, 0, 2]))
        wdef = const.tile([P, K * P], BF16, tag="wdef", name="wdef")
        nc.sync.dma_start(_r3(wdef, P), wdef_d[:, :, :].transpose([1, 0, 2]))
        wfus = const.tile([P, 2 * P], BF16, tag="wfus", name="wfus")
        nc.sync.dma_start(_r3(wfus, P), wfus_d[:, :, :].transpose([1, 0, 2]))
        bfus = const.tile([P, 1], F32, tag="bfus", name="bfus")
        nc.sync.dma_start(bfus[:, :], bfus_d[:, :])
        yb = const.tile([P, K * 32], F32, tag="yb", name="yb")
        nc.sync.dma_start(yb[:, :], yb_d[:, :])
        xb = const.tile([P, K * 32], F32, tag="xb", name="xb")
        nc.sync.dma_start(xb[:, :], xb_d[:, :])
        idn = const.tile([P, P], BF16, tag="idn", name="idn")
        nc.sync.dma_start(idn[:, :], idn_d[:, :])
        bm = const.tile([P, 13], F32, tag="bm", name="bm")
        nc.sync.dma_start(bm[:, :], bm_d[:, :])

        # ---- long-lived tiles ----
        xp = main.tile([P, XPW], BF16, tag="xp", name="xp")
        xstd = main.tile([P, NP], BF16, tag="xstd", name="xstd")
        # pmT halves: [p, b(16), ch(128)], ch>=32 garbage (never read)
        pmT = [main.tile([P, WBLK * P], BF16, tag=f"pmT{h}", name=f"pmT{h}")
               for h in range(2)]
        omb = [main.tile([P, WBLK * P], BF16, tag=f"omb{h}", name=f"omb{h}")
               for h in range(2)]  # [128, 2048]; rows 0..31 valid
        idx32 = main.tile([P, K * 32], mybir.dt.int32, tag="idx32", name="idx32")
        # corner scales, corner-interleaved + pair-duplicated:
        # col = (k*32+b)*8 + corner*2 + {0,1}; corner order matches the
        # gathered patch layout [v00 | v10 | v01 | v11].
        sall = main.tile([P, K * 32 * 8], BF16, tag="sall", name="sall")
        xdef = main.tile([P, NP], BF16, tag="xdef", name="xdef")

        def conv_rhs(n, ki, kj):
            base = (8 * n + ki + 1) * HP
            v = xp[:, base : base + 8 * HP]
            return _r3(v, HP)[:, :, kj + 1 : kj + 1 + H]

        # ================= phase A: load + padded bf16 image ============
        xpv = _r3(xp[:, 0:NPAD], HP)  # [128, 68, 68]
        if True:
            nc.vector.memset(xp[:, 0 : 2 * HP], 0.0)
            nc.vector.memset(xp[:, 66 * HP : NPAD], 0.0)
            nc.vector.memset(xpv[:, 2:66, 0:2], 0.0)
            nc.vector.memset(xpv[:, 2:66, 66:68], 0.0)
            nc.vector.memset(xp[:, NPAD:XPW], 0.0)
            nc.vector.tensor_copy(xpv[:, 2:66, 2:66], _r3(x_sb[:, :], H))

            # ========== phase B: channel-last image + row-pair DRAM =====
            xcl = pa.tile([P, NBLK, P], BF16, tag="xcl", name="xcl")
            # xcl[j, b, i] = xp[i, b*128 + j]  (one XBAR transpose)
            nc.sync.dma_start(xcl[:, :, :], xp[:, :], transpose=True)
            x2r = x2_d[:, :].rearrange("(b p) c -> p b c", p=P)  # [128, 37, 256]
            xclr = xcl[:, :, :]
            # plane 0: entry (r,c) <- x_cl[r,c]
            nc.sync.dma_start(x2r[:, :, 0:P], xclr)
            # plane 1: entry (r,c) <- x_cl[r+1,c]  (source shifted by 68 pos)
            nc.sync.dma_start(x2r[0:60, :, P:256], xclr[68:128, :, :])
            nc.sync.dma_start(x2r[60:128, 0 : NBLK - 1, P:256], xclr[0:68, 1:NBLK, :])
            pa_cm.__exit__(None, None, None)

        # ===== phases C/D/E per half: om conv -> transpose -> math ======
        with tc.tile_pool(name="ps_om", bufs=1, space="PSUM") as ps_om, \
             tc.tile_pool(name="ph_e", bufs=1) as pe_pool:

            def t288(tag, dt=F32):
                return pe_pool.tile([P, K * 32], dt, tag=tag, name=tag)

            py = t288("py"); px = t288("px")
            iy = t288("iy"); ix = t288("ix")
            wy = t288("wy"); wx = t288("wx")
            u = t288("u"); vv = t288("vv")
            a = t288("a"); bw = t288("bw")
            m = t288("m")
            idxf = t288("idxf")
            sg = pe_pool.tile([P, 13 * 32], F32, tag="sg", name="sg")
            ss = pe_pool.tile([P, 13 * 32], F32, tag="ss", name="ss")
            sall8 = sall.rearrange("p (kb e) -> p kb e", e=8)  # [128, 288, 8]

            for h in range(2):
                # ---- C: om conv for n-tiles of this half --------------
                ps = ps_om.tile([32, 4 * 512], F32, tag="om_ps", name="om_ps")
                for nl, n in enumerate(range(4 * h, 4 * h + 4)):
                    for k in range(K):
                        nc.tensor.matmul(
                            ps[:, nl * 512 : (nl + 1) * 512],
                            wom[:, k * 32 : (k + 1) * 32],
                            conv_rhs(n, k // 3, k % 3),
                            start=(k == 0), stop=(k == K - 1),
                        )
                nc.scalar.activation(omb[h][:32, :], ps[:, :], AF.Copy)
                # ---- D: one XBAR transpose to position-major ----------
                # pmT[h][j, b, ch] = omb[h][ch, b*128+j]
                nc.sync.dma_start(_r3(pmT[h], P), omb[h][:, :], transpose=True)

                # ---- E: index & weight math for this half's 16 blocks -
                pm3 = _r3(pmT[h], P)                          # [128, 16, 128]
                dyv = pm3[:, :, 0:18:2].transpose([0, 2, 1])  # [p, 9, 16]
                dxv = pm3[:, :, 1:19:2].transpose([0, 2, 1])
                sgin = pm3[:, :, 18:31].transpose([0, 2, 1])  # [p, 13, 16]
                bs = slice(h * WBLK, (h + 1) * WBLK)
                v3 = lambda t: _r3(t, 32)[:, :, bs]  # [128, 9, 16] slice

                nc.vector.tensor_tensor(v3(py), dyv, v3(yb), op=OP.add)
                nc.vector.tensor_tensor(v3(px), dxv, v3(xb), op=OP.add)
                for t in (py, px):
                    nc.vector.tensor_scalar(
                        v3(t), v3(t), 66.4, 0.6, op0=OP.min, op1=OP.max
                    )
                # floor via round-to-nearest(v - 0.5)
                nc.vector.tensor_scalar(v3(iy), v3(py), 0.5, MAGIC, op0=OP.subtract, op1=OP.add)
                nc.vector.tensor_scalar(v3(iy), v3(iy), MAGIC, None, op0=OP.subtract)
                nc.vector.tensor_scalar(v3(ix), v3(px), 0.5, MAGIC, op0=OP.subtract, op1=OP.add)
                nc.vector.tensor_scalar(v3(ix), v3(ix), MAGIC, None, op0=OP.subtract)
                nc.vector.tensor_tensor(v3(wy), v3(py), v3(iy), op=OP.subtract)
                nc.vector.tensor_tensor(v3(wx), v3(px), v3(ix), op=OP.subtract)
                # gather index = iy*68 + ix
                nc.vector.tensor_scalar(v3(idxf), v3(iy), 68.0, None, op0=OP.mult)
                nc.vector.tensor_tensor(v3(idxf), v3(idxf), v3(ix), op=OP.add)
                nc.vector.tensor_copy(_r3(idx32, 32)[:, :, bs], v3(idxf))

                # mask: sigmoid(std_mod + b_mod) * sigmoid(corner; absent -> 0.5)
                ssv = _r3(ss, 32)[:, :, bs]
                nc.vector.tensor_tensor(
                    ssv, sgin, bm[:, :].unsqueeze(2).broadcast_to((P, 13, WBLK)),
                    op=OP.add)
                sgv = _r3(sg, 32)[:, :, bs]
                nc.scalar.activation(sgv, ssv, AF.Sigmoid)
                sgr = _r3(sg, 32)
                for ci, k in enumerate((0, 2, 6, 8)):
                    nc.vector.tensor_tensor(
                        m[:, k * 32 + h * WBLK : k * 32 + h * WBLK + WBLK],
                        sgr[:, k, bs], sgr[:, 9 + ci, bs], op=OP.mult,
                    )
                for k in (1, 3, 4, 5, 7):
                    nc.vector.tensor_scalar(
                        m[:, k * 32 + h * WBLK : k * 32 + h * WBLK + WBLK],
                        sgr[:, k, bs], 0.5, None, op0=OP.mult,
                    )

                # corner scales (mask folded) into sall, pair-duplicated
                nc.vector.tensor_scalar(v3(u), v3(wy), -1.0, 1.0, op0=OP.mult, op1=OP.add)
                nc.vector.tensor_scalar(v3(vv), v3(wx), -1.0, 1.0, op0=OP.mult, op1=OP.add)
                nc.vector.tensor_tensor(v3(a), v3(m), v3(u), op=OP.mult)
                nc.vector.tensor_tensor(v3(bw), v3(m), v3(wy), op=OP.mult)
                s8 = sall8.rearrange("p (k b) e -> p k b e", k=K)[:, :, bs, :]
                pair = lambda t: _r3(t, 32)[:, :, bs].unsqueeze(3).broadcast_to(
                    (P, K, WBLK, 2))
                # corner order = patch layout [v00 | v10 | v01 | v11]
                nc.vector.tensor_tensor(s8[:, :, :, 0:2], pair(a), pair(vv), op=OP.mult)
                nc.vector.tensor_tensor(s8[:, :, :, 2:4], pair(bw), pair(vv), op=OP.mult)
                nc.vector.tensor_tensor(s8[:, :, :, 4:6], pair(a), pair(wx), op=OP.mult)
                nc.vector.tensor_tensor(s8[:, :, :, 6:8], pair(bw), pair(wx), op=OP.mult)

        # ============== std conv (overlaps gather ramp) =================
        with tc.tile_pool(name="ps_std", bufs=1, space="PSUM") as ps_std:
            for hh in range(2):
                pss = ps_std.tile([P, 2048], F32, tag="std_ps", name="std_ps")
                for nl, n in enumerate(range(4 * hh, 4 * hh + 4)):
                    for k in range(K):
                        nc.tensor.matmul(
                            pss[:, nl * 512 : (nl + 1) * 512],
                            wstd[:, k * P : (k + 1) * P],
                            conv_rhs(n, k // 3, k % 3),
                            start=(k == 0), stop=(k == K - 1),
                        )
                nc.vector.tensor_copy(
                    xstd[:, hh * 2048 : (hh + 1) * 2048], pss[:, :])

        # ========== phase G: gather + combine + def conv + fuse =========
        with tc.tile_pool(name="gpool", bufs=8) as gpool, \
             tc.tile_pool(name="qpool", bufs=3) as qpool, \
             tc.tile_pool(name="tpool", bufs=2) as tp1, \
             tc.tile_pool(name="spool", bufs=4) as spool, \
             tc.tile_pool(name="rpool", bufs=4) as rpool, \
             tc.tile_pool(name="ph_h", bufs=2) as ph, \
             tc.tile_pool(name="ps_fus", bufs=2, space="PSUM") as ps_fus, \
             tc.tile_pool(name="ps_tr", bufs=2, space="PSUM") as ps_tr, \
             tc.tile_pool(name="ps_def", bufs=1, space="PSUM", side="right") as ps_def:

            def chunk(w, k, cb0, ncb, psd):
                """Process blocks [cb0, cb0+ncb) of wave w, tap k."""
                c0 = k * 32 + w * WBLK + cb0
                g = gpool.tile([P, 8, 512], BF16, tag="g", name="g")
                g = g[:, 0:ncb, :]
                for bb in range(ncb):
                    nc.gpsimd.indirect_dma_start(
                        out=g[:, bb, :],
                        out_offset=None,
                        in_=x2_d[:, :],
                        in_offset=bass.IndirectOffsetOnAxis(
                            ap=idx32[:, c0 + bb : c0 + bb + 1], axis=0
                        ),
                    )
                # combine: q = g * scales (2x), then 4:1 tree add (2x)
                q = qpool.tile([P, 8 * 512], BF16, tag="q", name="q")[:, 0 : ncb * 512]
                t = tp1.tile([P, 8 * 256], BF16, tag="t", name="t")[:, 0 : ncb * 256]
                samp = spool.tile([P, 8 * P], BF16, tag="samp", name="samp")[:, 0 : ncb * P]
                gv = g[:, :, :].rearrange("p b (c x e) -> p (b c) x e", c=4, e=2)
                sv = (
                    sall[:, c0 * 8 : (c0 + ncb) * 8]
                    .rearrange("p (bc o e) -> p bc o e", o=1, e=2)
                    .broadcast_to((P, ncb * 4, 64, 2))
                )
                qv = q.rearrange("p (bc x e) -> p bc x e", bc=ncb * 4, e=2)
                nc.vector.tensor_tensor(qv, gv, sv, op=OP.mult)
                qh = q.rearrange("p (b h) -> p b h", h=512)
                th = t.rearrange("p (b h) -> p b h", h=256)
                nc.vector.tensor_tensor(th, qh[:, :, 0:256], qh[:, :, 256:512], op=OP.add)
                sh = samp.rearrange("p (b h) -> p b h", h=128)
                nc.vector.tensor_tensor(sh, th[:, :, 0:128], th[:, :, 128:256], op=OP.add)

                # rhsT[ch, bb*128+pos] = samp[pos, bb*128+ch] (PE transposes)
                rhsT = rpool.tile([P, 8, P], BF16, tag="rhsT", name="rhsT")
                rhsT = rhsT[:, 0:ncb, :]
                for bb in range(ncb):
                    tp = ps_tr.tile([P, 256], BF16, tag="tp", name="tp")
                    nc.tensor.transpose(
                        tp[:, :P], samp[:, bb * P : (bb + 1) * P], idn
                    )
                    nc.scalar.activation(rhsT[:, bb, :], tp[:, :P], AF.Copy)
                for bb in range(ncb):
                    gb = cb0 + bb
                    # start marks the whole 2KB PSUM bank (4 blocks)
                    # pending-zero, so only the first block of each bank
                    # may set it.
                    nc.tensor.matmul(
                        psd[:, gb * P : (gb + 1) * P],
                        wdef[:, k * P : (k + 1) * P],
                        rhsT[:, bb, :],
                        start=(k == 0 and gb % 4 == 0),
                        stop=(k == K - 1 and gb % 4 == 3),
                        skip_group_check=True,
                    )

            for w in range(NW):
                psd = ps_def.tile([P, WBLK * P], F32, tag="psd", name="psd")
                for k in range(K - 1):
                    chunk(w, k, 0, 8, psd)
                    chunk(w, k, 8, 8, psd)
                ncl = 4 if w == NW - 1 else 8
                for cb0 in range(0, WBLK, ncl):
                    chunk(w, K - 1, cb0, ncl, psd)
                    if cb0 + ncl == 8 or cb0 + ncl == WBLK:
                        lo = 0 if cb0 + ncl == 8 else 8
                        nc.vector.tensor_copy(
                            xdef[:, (w * WBLK + lo) * P : (w * WBLK + lo + 8) * P],
                            psd[:, lo * P : (lo + 8) * P])
                # ====== fused 1x1 conv for this wave's 4 n-tiles ========
                for n in range(w * 4, w * 4 + 4):
                    ps = ps_fus.tile([P, 512], F32, tag="ps_h", name="ps_h")
                    nc.tensor.matmul(
                        ps[:, :], wfus[:, 0:P], xstd[:, n * 512 : (n + 1) * 512],
                        start=True, stop=False,
                    )
                    nc.tensor.matmul(
                        ps[:, :], wfus[:, P : 2 * P], xdef[:, n * 512 : (n + 1) * 512],
                        start=False, stop=True,
                    )
                    stage = ph.tile([P, 512], F32, tag="stage", name="stage")
                    nc.scalar.activation(stage[:, :], ps[:, :], AF.Identity, bias=bfus[:, :])
                    nc.sync.dma_start(out_d[:, n * 512 : (n + 1) * 512], stage[:, :])

    return nc


def _consts(W_std, b_std, W_off, b_off, W_mod, b_mod, W_def, b_def, W_fus, b_fus):
    """Host-side constant prep (shared across cores)."""
    f = np.float32
    wstd = np.transpose(W_std, (2, 3, 1, 0)).reshape(K, P, P)  # [k, c, o]
    wom_full = np.concatenate([W_off, W_mod], axis=0)  # [31, 128, 3, 3]
    wom = np.zeros((K, P, 32), f)
    wom[:, :, :31] = np.transpose(wom_full, (2, 3, 1, 0)).reshape(K, P, 31)
    wdef = np.transpose(W_def, (2, 3, 1, 0)).reshape(K, P, P)
    wf = W_fus[:, :, 0, 0]  # [128, 256]
    wfus = np.stack([wf[:, :P].T, wf[:, P:].T], axis=0)  # [2, c, o]
    bfus = (b_fus + wf[:, :P] @ b_std + wf[:, P:] @ b_def).reshape(P, 1)
    # ybase/xbase in [p, k*32+b] layout: j = b*128 + p; conv bias folded in
    pp, kk, bb2 = np.meshgrid(np.arange(P), np.arange(K), np.arange(32), indexing="ij")
    j = bb2 * 128 + pp
    yb = ((j >> 6) + (kk // 3) + 1 + b_off[2 * kk]).astype(f).reshape(P, K * 32)
    xb = ((j & 63) + (kk % 3) + 1 + b_off[2 * kk + 1]).astype(f).reshape(P, K * 32)
    bm = np.broadcast_to(b_mod[None, :], (P, 13)).astype(f).copy()
    return dict(
        wstd=wstd.astype(NPBF), wom=wom.astype(NPBF), wdef=wdef.astype(NPBF),
        wfus=wfus.astype(NPBF), bfus=bfus.astype(f), bm=bm,
        yb=yb, xb=xb, idn=np.eye(P, dtype=NPBF),
    )


_NC_CACHE = {}


def _get_nc():
    if "nc" not in _NC_CACHE:
        nc = build_nc()
        nc.finalize()
        _NC_CACHE["nc"] = nc
    return _NC_CACHE["nc"]


def kernel(x, W_std, b_std, W_off, b_off, W_corner, b_corner, W_mod, b_mod,
           W_def, b_def, W_fus, b_fus, **kw):
    consts = _consts(
        np.asarray(W_std, np.float32), np.asarray(b_std, np.float32),
        np.asarray(W_off, np.float32), np.asarray(b_off, np.float32),
        np.asarray(W_mod, np.float32), np.asarray(b_mod, np.float32),
        np.asarray(W_def, np.float32), np.asarray(b_def, np.float32),
        np.asarray(W_fus, np.float32), np.asarray(b_fus, np.float32),
    )
    x = np.asarray(x, np.float32)
    B = x.shape[0]
    assert B == N_CORES, x.shape
    in_maps = []
    for b in range(B):
        im = dict(consts)
        im["x"] = np.ascontiguousarray(x[b].reshape(P, NP))
        in_maps.append(im)
    nc = _get_nc()
    res = run_bass_kernel_spmd(nc, in_maps, core_ids=list(range(N_CORES)))
    out = np.stack([r["out"].reshape(P, H, H) for r in res.results], axis=0)
    return out.astype(np.float32)


if __name__ == "__main__":
    nc = build_nc()
    nc.finalize()
    print("built ok")
